# revision 32
# baseline (speedup 1.0000x reference)
"""Trainium2 Bass kernel for the AlignmentWithPrototype (vq_codebook) problem.

Math (matching the reference):
  x = projections.reshape(N, D); xn = l2norm(x); pn = l2norm(concat(lP, gP))
  sim = xn @ pn.T ; E = exp(sim/T)
  Sinkhorn(3 iters) keeps Q = E * a_n * b_k:
      a_n = 1/(N * sum_k E[n,k] b_k) ; b_k = 1/(K * sum_n E[n,k] a_n)  (K=2M=256)
  local/global assign = argmax_k E[n,k]*b_k over each half (a_n is row-constant)
  memory = gP[g_assign]; lin = memory @ W + bias; gated = lin_a * sigmoid(lin_b)
  out = l2norm(0.5*gated + 0.5*x)  == l2norm(gated + x)   (scale invariant)
  new_local = EMA last-write-wins scatter (done on host from local_assign)

Sharding: data-parallel over tokens, 8 cores x 16384 tokens. Column sums of the
Sinkhorn need a tiny (256 floats) AllReduce per iteration.

Precision choices: sim and E stay fp32 (argmax discrimination); the E^T spill
(row-sum side), the Sinkhorn `a` vector, and the whole GLU path run in bf16 —
their errors either average out (a, cs) or are far below output tolerance.
"""

import numpy as np

import concourse.bacc as bacc
import concourse.bass as bass
import concourse.mybir as mybir
import concourse.tile as tile


def _patch_act_tables():
    """Restrict the activation-table chooser to two sets (their positional
    ids are preserved; the others are emptied so the first-match chooser
    can't bounce between per-function sets, which costs ~1.3us per load)."""
    import concourse.hw_specs as hw_specs
    orig = hw_specs.get_activation_tables
    keep = {"natural_log_exp_and_others", "sigmoid_and_others"}

    def patched(module_arch):
        full = orig(module_arch)
        return {k: (v if k in keep else set()) for k, v in full.items()}

    bacc.get_activation_tables = patched


_patch_act_tables()

F32 = mybir.dt.float32
BF16 = mybir.dt.bfloat16
U32 = mybir.dt.uint32
U16 = mybir.dt.uint16
AF = mybir.ActivationFunctionType
ALU = mybir.AluOpType

D = 192
M = 128
K2 = 256  # 2*M
TEMP = 0.05
DECAY = 0.96
N_CORES = 8


def _pbcast(ap, p=128):
    """Broadcast a [1, F...] DRAM AP across p partitions (stride-0 partition
    dim; only legal as a DMA source)."""
    return bass.AP(tensor=ap.tensor, offset=ap.offset, ap=[[0, p]] + list(ap.ap[1:]))


def build_nc(N_s: int, with_bias: bool, n_cores: int = N_CORES):
    assert N_s % 512 == 0
    NT = N_s // 128      # token tiles
    NCH = N_s // 512     # chunks
    CPT = 4              # tiles per chunk
    N_total = N_s * n_cores

    nc = bacc.Bacc("TRN2", target_bir_lowering=False, debug=False,
                   num_devices=n_cores)

    x_d = nc.dram_tensor("x", [N_s, D], F32, kind="ExternalInput")
    pnT_d = nc.dram_tensor("pnT", [128, 2, K2], F32, kind="ExternalInput")
    pnT16_d = nc.dram_tensor("pnT16", [128, 2, K2], BF16, kind="ExternalInput")
    gP16_d = nc.dram_tensor("gP16", [128, K2], BF16, kind="ExternalInput")
    gluW16_d = nc.dram_tensor("gluW16", [128, 2, 2 * D], BF16, kind="ExternalInput")
    glub_d = nc.dram_tensor("glub", [1, 2 * D], F32, kind="ExternalInput")
    ident_d = nc.dram_tensor("ident", [128, 128], F32, kind="ExternalInput")
    iota_d = nc.dram_tensor("iota", [128, 1], F32, kind="ExternalInput")

    y_d = nc.dram_tensor("y", [N_s, D], F32, kind="ExternalOutput")
    lidx_d = nc.dram_tensor("lidx", [N_s], F32, kind="ExternalOutput")
    gidx_d = nc.dram_tensor("gidx", [N_s], F32, kind="ExternalOutput")

    ET_d = nc.dram_tensor("ET_spill", [K2, N_s], BF16)
    rs_d = nc.dram_tensor("rs_bounce", [N_s], F32)
    g_d = nc.dram_tensor("g_bounce", [N_s], F32)
    b_d = nc.dram_tensor("b_bounce", [1, K2], F32)
    cc_in = nc.dram_tensor("cc_in", [1, K2], F32)
    cc_out = nc.dram_tensor("cc_out", [n_cores, K2], F32, addr_space="Shared")
    V_d = nc.dram_tensor("V_spill", [N_s, D], F32)

    x_t = x_d[:].rearrange("(t p) d -> p t d", p=128)   # [128, NT, 192]
    y_t = y_d[:].rearrange("(t p) d -> p t d", p=128)
    v_t = V_d[:].rearrange("(t p) d -> p t d", p=128)

    with tile.TileContext(nc) as tc:
        with (
            tc.tile_pool(name="consts", bufs=1) as consts,
            tc.tile_pool(name="small", bufs=1) as small,
            tc.tile_pool(name="io", bufs=3) as io,
        ):
            pnT = consts.tile([128, 2, K2], F32)
            nc.sync.dma_start(pnT, pnT_d[:])
            pnT16 = consts.tile([128, 2, K2], BF16)
            nc.sync.dma_start(pnT16, pnT16_d[:])
            gP16 = consts.tile([128, K2], BF16)
            nc.sync.dma_start(gP16, gP16_d[:])
            gluW16 = consts.tile([128, 2, 2 * D], BF16)
            nc.sync.dma_start(gluW16, gluW16_d[:])
            glub = consts.tile([128, 2 * D], F32)
            nc.sync.dma_start(glub, _pbcast(glub_d[:]))
            ident = consts.tile([128, 128], F32)
            nc.sync.dma_start(ident, ident_d[:])
            iota = consts.tile([128, 1], F32)
            nc.sync.dma_start(iota, iota_d[:])
            ones8 = consts.tile([n_cores, 1], F32)
            nc.vector.memset(ones8, 1.0)
            ones1r = consts.tile([1, 128], F32)
            nc.vector.memset(ones1r, 1.0)

            ss = small.tile([128, NT], F32)       # row sum-of-squares
            rn = small.tile([128, NT], F32)       # 1/sqrt(ss)
            acol = small.tile([128, NT], F32)     # sinkhorn a (column layout)
            acol16 = small.tile([128, NT], BF16)
            rcol = small.tile([128, NT], F32)     # raw row sums (column layout)
            lcols = small.tile([128, NT], F32)    # local argmax (col layout)
            gcols = small.tile([128, NT], F32)
            vss = small.tile([128, NT], F32)      # output sumsq
            b128 = small.tile([128, K2], F32)     # final b bcast across partitions
            bcol16 = small.tile([128, 2, 1], BF16)
            ixl = small.tile([128, NT, 8], U32)   # top-8 local argmax indices
            ixg = small.tile([128, NT, 8], U32)   # top-8 global argmax indices
            magic = small.tile([128, 1], U32)
            nc.vector.memset(magic, 0x5f3759df)

            with tc.tile_pool(name="Epool", bufs=1) as Epool:
                E = Epool.tile([128, NT, K2], F32)

                # ------- Phase 1: normalize+transpose, sim, E, E^T ----------
                # iteration-1 Sinkhorn column sums accumulate during phase 1
                # (row sums with b=1 fall out of the exp accumulators)
                psCS_cm = tc.tile_pool(name="psCS", bufs=1, space="PSUM")
                psCS = psCS_cm.__enter__()
                ps_cs1 = psCS.tile([1, K2], F32, tag="cs1")
                with (
                    tc.tile_pool(name="p1w", bufs=3) as p1w,
                    tc.tile_pool(name="psT", bufs=1, space="PSUM") as psT,
                    tc.tile_pool(name="psS", bufs=2, space="PSUM") as psS,
                    tc.tile_pool(name="psX", bufs=1, space="PSUM") as psX,
                ):
                    pt_lo = psT.tile([128, 512], F32, tag="ptlo")
                    pt_hi = psT.tile([128, 512], F32, tag="pthi")
                    nc.vector.memset(pt_hi[64:128, :], 0.0)
                    for c in range(NCH):
                        xt = io.tile([128, CPT, D], F32, tag="xnat")
                        nc.sync.dma_start(xt, x_t[:, c * CPT:(c + 1) * CPT, :])
                        csl = slice(c * CPT, (c + 1) * CPT)
                        for t in range(CPT):
                            tt = c * CPT + t
                            scr = p1w.tile([128, D], F32, tag="scr")
                            nc.scalar.activation(
                                scr, xt[:, t, :], AF.Square,
                                accum_out=ss[:, tt:tt + 1],
                            )
                        # rn = exp(-0.5*ln(ss)) = 1/sqrt(ss)
                        nc.scalar.activation(rn[:, csl], ss[:, csl], AF.Ln)
                        nc.scalar.activation(rn[:, csl], rn[:, csl], AF.Exp,
                                             scale=-0.5)
                        # transpose+normalize: xT[:, tok] = x_tile.T @ diag(rn)
                        for t in range(CPT):
                            tt = c * CPT + t
                            dg = p1w.tile([128, 128], F32, tag="dg")
                            nc.vector.tensor_tensor(
                                dg, ident, rn[:, tt:tt + 1].to_broadcast((128, 128)),
                                op=ALU.mult,
                            )
                            nc.tensor.matmul(
                                pt_lo[:, t * 128:(t + 1) * 128], xt[:, t, 0:128], dg,
                                start=True, stop=True,
                            )
                            nc.tensor.matmul(
                                pt_hi[0:64, t * 128:(t + 1) * 128], xt[:, t, 128:192], dg,
                                start=True, stop=True,
                            )
                        xT = p1w.tile([128, 2, 512], F32, tag="xT")
                        nc.vector.tensor_copy(xT[:, 0, :], pt_lo)
                        nc.vector.tensor_copy(xT[:, 1, :], pt_hi)
                        xT16 = p1w.tile([128, 2, 512], BF16, tag="xT16")
                        nc.vector.tensor_copy(xT16[:, 0, :], pt_lo)
                        nc.vector.tensor_copy(xT16[:, 1, :], pt_hi)
                        # sim (token-major, fp32) -> E
                        for t in range(CPT):
                            tt = c * CPT + t
                            ps = psS.tile([128, K2], F32, tag="sim")
                            nc.tensor.matmul(
                                ps, xT[:, 0, t * 128:(t + 1) * 128], pnT[:, 0, :],
                                start=True, stop=False,
                            )
                            nc.tensor.matmul(
                                ps, xT[:, 1, t * 128:(t + 1) * 128], pnT[:, 1, :],
                                start=False, stop=True,
                            )
                            nc.scalar.activation(
                                E[:, tt, :], ps, AF.Exp, scale=1.0 / TEMP,
                                accum_out=rcol[:, tt:tt + 1],
                            )
                        # iteration-1 a-columns and cs accumulation
                        nc.vector.tensor_scalar_mul(
                            acol[:, csl], rcol[:, csl], float(N_total)
                        )
                        nc.vector.reciprocal(acol[:, csl], acol[:, csl])
                        nc.vector.tensor_copy(acol16[:, csl], acol[:, csl])
                        E16v_ = E.bitcast(BF16)
                        for t in range(CPT):
                            tt = c * CPT + t
                            nc.tensor.matmul(
                                ps_cs1, acol16[:, tt:tt + 1], E16v_[:, tt, 1::2],
                                start=(tt == 0), stop=(tt == NT - 1),
                            )
                        # simT (proto-major, bf16) -> E^T (bf16) -> DRAM
                        ps1 = psX.tile([128, 512], F32, tag="sT1")
                        ps2 = psX.tile([128, 512], F32, tag="sT2")
                        nc.tensor.matmul(ps1, pnT16[:, 0, 0:128], xT16[:, 0, :],
                                         start=True, stop=False)
                        nc.tensor.matmul(ps1, pnT16[:, 1, 0:128], xT16[:, 1, :],
                                         start=False, stop=True)
                        nc.tensor.matmul(ps2, pnT16[:, 0, 128:256], xT16[:, 0, :],
                                         start=True, stop=False)
                        nc.tensor.matmul(ps2, pnT16[:, 1, 128:256], xT16[:, 1, :],
                                         start=False, stop=True)
                        et16 = p1w.tile([128, 2, 512], BF16, tag="et16")
                        nc.scalar.activation(et16[:, 0, :], ps1, AF.Exp, scale=1.0 / TEMP)
                        nc.scalar.activation(et16[:, 1, :], ps2, AF.Exp, scale=1.0 / TEMP)
                        sl = slice(c * 512, (c + 1) * 512)
                        nc.sync.dma_start(ET_d[0:128, sl], et16[:, 0, :])
                        nc.sync.dma_start(ET_d[128:256, sl], et16[:, 1, :])

                # ---------------- Phase 2: Sinkhorn iterations -------------
                nc.vector.memset(bcol16, 1.0)
                with (
                    tc.tile_pool(name="p2w", bufs=3) as p2w,
                    tc.tile_pool(name="etp", bufs=8) as etp,
                    tc.tile_pool(name="psR", bufs=3, space="PSUM") as psR,
                    tc.tile_pool(name="psC", bufs=1, space="PSUM") as psC,
                ):
                    N_ITERS = 2
                    for it in range(N_ITERS):
                        if it > 0:
                            # row sums with current b (iteration-1 rs and cs
                            # were already folded into phase 1)
                            for c in range(NCH):
                                et = etp.tile([128, 2, 512], BF16, tag="et")
                                sl = slice(c * 512, (c + 1) * 512)
                                sl_a = slice(c * 512, c * 512 + 256)
                                sl_b = slice(c * 512 + 256, (c + 1) * 512)
                                eng0 = [nc.sync, nc.scalar][c % 2]
                                eng1 = [nc.scalar, nc.sync][c % 2]
                                eng0.dma_start(et[:, 0, 0:256], ET_d[0:128, sl_a])
                                eng1.dma_start(et[:, 0, 256:512], ET_d[0:128, sl_b])
                                eng1.dma_start(et[:, 1, 0:256], ET_d[128:256, sl_a])
                                eng0.dma_start(et[:, 1, 256:512], ET_d[128:256, sl_b])
                                ps_rs = psR.tile([1, 512], F32, tag="rs")
                                nc.tensor.matmul(
                                    ps_rs, bcol16[:, 0, :], et[:, 0, :],
                                    start=True, stop=False,
                                )
                                nc.tensor.matmul(
                                    ps_rs, bcol16[:, 1, :], et[:, 1, :],
                                    start=False, stop=True,
                                )
                                rsr = p2w.tile([1, 512], F32, tag="rsr")
                                nc.scalar.copy(rsr, ps_rs)
                                nc.sync.dma_start(rs_d[sl], rsr)
                            # read back in column layout [p, t] (strided
                            # gather, split across DMA queues)
                            rs_v = rs_d[:].rearrange("(t p) -> p t", p=128)
                            q4 = NT // 4
                            for qq in range(4):
                                nc.sync.dma_start(
                                    rcol[:, qq * q4:(qq + 1) * q4],
                                    rs_v[:, qq * q4:(qq + 1) * q4],
                                )
                        if it == 0:
                            ps_cs = ps_cs1
                        else:
                            # a = 1/(N_total * rs)
                            nc.vector.tensor_scalar_mul(acol, rcol, float(N_total))
                            nc.vector.reciprocal(acol, acol)
                            nc.vector.tensor_copy(acol16, acol)
                            # cs[k] = sum_n a_n E[n,k] in bf16: read the high
                            # half-words of fp32 E as a strided bf16 view
                            # (truncation bias is uniform and cancels in b)
                            E16v = E.bitcast(BF16)
                            ps_cs = psC.tile([1, K2], F32, tag="cs")
                            for tt in range(NT):
                                nc.tensor.matmul(
                                    ps_cs, acol16[:, tt:tt + 1],
                                    E16v[:, tt, 1::2],
                                    start=(tt == 0), stop=(tt == NT - 1),
                                )
                        csrow = p2w.tile([1, K2], F32, tag="csrow")
                        nc.vector.tensor_copy(csrow, ps_cs)
                        nc.sync.dma_start(cc_in[:], csrow)
                        nc.gpsimd.collective_compute(
                            "AllGather",
                            ALU.bypass,
                            replica_groups=[list(range(n_cores))],
                            ins=[cc_in[:]],
                            outs=[cc_out[:]],
                        )
                        csg = p2w.tile([n_cores, K2], F32, tag="csg")
                        nc.sync.dma_start(csg, cc_out[:])
                        ps_bs = psC.tile([1, K2], F32, tag="bs")
                        nc.tensor.matmul(ps_bs, ones8, csg, start=True, stop=True)
                        bsb = p2w.tile([1, K2], F32, tag="bsb")
                        nc.vector.tensor_scalar_mul(bsb, ps_bs, float(K2))
                        nc.vector.reciprocal(bsb, bsb)
                        if it < N_ITERS - 1:
                            # b columns via PE transpose of the two halves
                            ps_bt = psC.tile([128, 2], F32, tag="bt")
                            nc.tensor.transpose(ps_bt[:, 0:1], bsb[:, 0:128],
                                                ones1r[:, 0:1])
                            nc.tensor.transpose(ps_bt[:, 1:2], bsb[:, 128:256],
                                                ones1r[:, 0:1])
                            nc.vector.tensor_copy(
                                bcol16, ps_bt[:].rearrange("p (c o) -> p c o", o=1)
                            )
                        else:
                            # b128 = ones ⊗ b via a rank-1 matmul broadcast
                            ps_bb = psC.tile([128, K2], F32, tag="bb")
                            nc.tensor.matmul(ps_bb, ones1r, bsb, start=True, stop=True)
                            nc.vector.tensor_copy(b128, ps_bb)

                psCS_cm.__exit__(None, None, None)

                # ------- Phase 3+4a fused: argmax -> gather -> GLU -> v -----
                with (
                    tc.tile_pool(name="p4w", bufs=3) as p4w,
                    tc.tile_pool(name="vtp", bufs=4) as vtp,
                    tc.tile_pool(name="psI", bufs=2, space="PSUM") as psI,
                    tc.tile_pool(name="psM", bufs=1, space="PSUM") as psM,
                    tc.tile_pool(name="psL", bufs=2, space="PSUM") as psL,
                ):
                    GRP = 1
                    for g in range(NCH // GRP):
                        vts = []
                        for c4 in range(GRP):
                            c = g * GRP + c4
                            csl4 = slice(c * CPT, (c + 1) * CPT)
                            for t in range(CPT):
                                tt = c * CPT + t
                                qt = p4w.tile([128, K2], F32, tag="qt")
                                nc.gpsimd.tensor_tensor(qt, E[:, tt, :], b128,
                                                        op=ALU.mult)
                                vm = p4w.tile([128, 8], F32, tag="vm")
                                nc.vector.max(vm, qt[:, 0:128])
                                nc.vector.max_index(ixl[:, tt, :], vm, qt[:, 0:128])
                                vm2 = p4w.tile([128, 8], F32, tag="vm2")
                                nc.vector.max(vm2, qt[:, 128:256])
                                nc.vector.max_index(ixg[:, tt, :], vm2, qt[:, 128:256])
                            # extract this chunk's gidx as f32 columns (strided)
                            nc.vector.tensor_copy(gcols[:, csl4],
                                                  ixg[:, csl4, 0])
                            # bounce this chunk's gidx to token-major row in DRAM
                            pig = psI.tile([CPT, 128], F32, tag="pig")
                            nc.tensor.transpose(pig, gcols[:, csl4], ident)
                            gTc = p4w.tile([CPT, 128], F32, tag="gTc")
                            nc.scalar.copy(gTc, pig)
                            gsl = slice(c * 512, (c + 1) * 512)
                            nc.sync.dma_start(
                                g_d[gsl].rearrange("(t p) -> t p", p=128), gTc
                            )
                            g128 = p4w.tile([128, 512], F32, tag="g128")
                            nc.sync.dma_start(
                                g128,
                                _pbcast(g_d[:].rearrange("(a f) -> a f", f=512)[c:c + 1, :]),
                            )
                            oh = p4w.tile([128, 512], BF16, tag="oh")
                            nc.vector.tensor_tensor(
                                oh, iota[:].to_broadcast((128, 512)), g128,
                                op=ALU.is_equal,
                            )
                            ps_ma1 = psM.tile([128, 512], F32, tag="ma1")
                            ps_ma2 = psM.tile([128, 512], F32, tag="ma2")
                            nc.tensor.matmul(ps_ma1, gP16[:, 0:128], oh, start=True, stop=True)
                            nc.tensor.matmul(ps_ma2, gP16[:, 128:256], oh, start=True, stop=True)
                            maT = p4w.tile([128, 2, 512], BF16, tag="maT")
                            nc.scalar.copy(maT[:, 0, :], ps_ma1)
                            nc.scalar.copy(maT[:, 1, :], ps_ma2)
                            x2 = io.tile([128, CPT, D], F32, tag="xnat")
                            nc.sync.dma_start(x2, x_t[:, c * CPT:(c + 1) * CPT, :])
                            vt = vtp.tile([128, CPT, D], F32, tag="vt")
                            vts.append(vt)
                            for t in range(CPT):
                                tt = c * CPT + t
                                tsl = slice(t * 128, (t + 1) * 128)
                                ps_lin = psL.tile([128, 2 * D], F32, tag="lin")
                                nc.tensor.matmul(ps_lin, maT[:, 0, tsl], gluW16[:, 0, :],
                                                 start=True, stop=False)
                                nc.tensor.matmul(ps_lin, maT[:, 1, tsl], gluW16[:, 1, :],
                                                 start=False, stop=True)
                                if with_bias:
                                    lsb = p4w.tile([128, 2 * D], F32, tag="lsb")
                                    nc.vector.tensor_tensor(
                                        lsb, ps_lin, glub, op=ALU.add
                                    )
                                    lin_src = lsb
                                else:
                                    lin_src = ps_lin
                                sg = p4w.tile([128, D], F32, tag="sg")
                                nc.scalar.activation(sg, lin_src[:, D:2 * D], AF.Sigmoid)
                                nc.vector.tensor_tensor(
                                    vt[:, t, :], lin_src[:, 0:D], sg, op=ALU.mult
                                )
                                nc.gpsimd.tensor_tensor(
                                    vt[:, t, :], vt[:, t, :], x2[:, t, :], op=ALU.add
                                )
                                scr2 = p4w.tile([128, D], F32, tag="scr2")
                                nc.scalar.activation(
                                    scr2, vt[:, t, :], AF.Square,
                                    accum_out=vss[:, tt:tt + 1],
                                )
                        # rnorm = rsqrt(vss) for the whole group:
                        # magic seed + 2 Newton steps (batched [128, 16])
                        gw = GRP * CPT
                        sgrp = vss[:, g * gw:(g + 1) * gw]
                        rg = p4w.tile([128, gw], F32, tag="rg")
                        rgu = rg.bitcast(U32)
                        nc.vector.tensor_scalar(
                            rgu, sgrp.bitcast(U32), 1, None,
                            op0=ALU.logical_shift_right,
                        )
                        nc.vector.tensor_tensor(
                            rgu, magic[:].to_broadcast((128, gw)), rgu,
                            op=ALU.subtract,
                        )
                        t1 = p4w.tile([128, gw], F32, tag="t1")
                        for _ in range(2):
                            nc.vector.tensor_tensor(t1, rg, rg, op=ALU.mult)
                            nc.vector.tensor_tensor(t1, t1, sgrp, op=ALU.mult)
                            nc.vector.tensor_scalar(
                                t1, t1, -0.5, 1.5, op0=ALU.mult, op1=ALU.add
                            )
                            nc.vector.tensor_tensor(rg, rg, t1, op=ALU.mult)
                        for c4 in range(GRP):
                            c = g * GRP + c4
                            yt = p4w.tile([128, CPT, D], F32, tag="yt")
                            for t in range(CPT):
                                nc.gpsimd.tensor_scalar_mul(
                                    yt[:, t, :], vts[c4][:, t, :],
                                    rg[:, c4 * CPT + t:c4 * CPT + t + 1]
                                )
                            nc.sync.dma_start(y_t[:, c * CPT:(c + 1) * CPT, :], yt)
                    # local argmax columns -> token-major output (host uses it)
                    nc.vector.tensor_copy(lcols, ixl[:, :, 0])
                    pil = psI.tile([NT, 128], F32, tag="pig")
                    nc.tensor.transpose(pil, lcols, ident)
                    lT = p4w.tile([NT, 128], F32, tag="lT")
                    nc.vector.tensor_copy(lT, pil)
                    nc.sync.dma_start(
                        lidx_d[:].rearrange("(t p) -> t p", p=128), lT
                    )
                    pig2 = psI.tile([NT, 128], F32, tag="pig")
                    nc.tensor.transpose(pig2, gcols, ident)
                    gT = p4w.tile([NT, 128], F32, tag="gT")
                    nc.vector.tensor_copy(gT, pig2)
                    nc.sync.dma_start(
                        gidx_d[:].rearrange("(t p) -> t p", p=128), gT
                    )


    nc.compile()
    return nc


def _host_consts(localPrototypes, globalPrototypes, glu_W, glu_b):
    protos = np.concatenate([localPrototypes, globalPrototypes], axis=0).astype(np.float32)
    sq = np.maximum((protos * protos).sum(axis=1, keepdims=True), 1e-12)
    pn = protos / np.sqrt(sq)

    pnT_pad = np.zeros((256, K2), np.float32)
    pnT_pad[0:D] = pn.T
    pnT_pad = np.ascontiguousarray(pnT_pad.reshape(2, 128, K2).transpose(1, 0, 2))

    import ml_dtypes
    bf16 = ml_dtypes.bfloat16

    gP_pad = np.zeros((128, K2), np.float32)
    gP_pad[:, 0:D] = globalPrototypes.astype(np.float32)

    gluW_pad = np.zeros((256, 2 * D), np.float32)
    gluW_pad[0:D] = glu_W.astype(np.float32)
    gluW_pad = np.ascontiguousarray(gluW_pad.reshape(2, 128, 2 * D).transpose(1, 0, 2))

    return {
        "pnT": pnT_pad,
        "pnT16": pnT_pad.astype(bf16),
        "gP16": gP_pad.astype(bf16),
        "gluW16": gluW_pad.astype(bf16),
        "glub": np.ascontiguousarray(glu_b.astype(np.float32).reshape(1, 2 * D)),
        "ident": np.eye(128, dtype=np.float32),
        "iota": np.arange(128, dtype=np.float32).reshape(128, 1),
    }


_NC_CACHE = {}


def _get_nc(N_s, with_bias):
    key = (N_s, with_bias)
    if key not in _NC_CACHE:
        _NC_CACHE[key] = build_nc(N_s, with_bias)
    return _NC_CACHE[key]


def kernel(projections, localPrototypes, globalPrototypes, glu_W, glu_b):
    from concourse.bass_utils import run_bass_kernel_spmd

    B, T, d = projections.shape
    N = B * T
    N_s = N // N_CORES
    x_flat = np.ascontiguousarray(projections.astype(np.float32).reshape(N, d))

    with_bias = bool(np.any(glu_b))
    consts = _host_consts(localPrototypes, globalPrototypes, glu_W, glu_b)
    nc = _get_nc(N_s, with_bias)

    in_maps = []
    for c in range(N_CORES):
        m = {"x": x_flat[c * N_s:(c + 1) * N_s]}
        m.update(consts)
        in_maps.append(m)

    res = run_bass_kernel_spmd(nc, in_maps, list(range(N_CORES)))
    results = res.results

    y = np.concatenate([r["y"] for r in results], axis=0)
    normed = y.reshape(B, T, d)

    lidx = np.concatenate([r["lidx"] for r in results], axis=0)
    lidx = np.rint(lidx).astype(np.int64)

    # EMA scatter, last-write-wins
    last = np.full(M, -1, dtype=np.int64)
    last[lidx] = np.arange(N, dtype=np.int64)
    lP = localPrototypes.astype(np.float32)
    new_local = lP.copy()
    valid = last >= 0
    new_local[valid] = DECAY * lP[valid] + np.float32(1.0 - DECAY) * x_flat[last[valid]]

    return normed, new_local


# revision 33
# speedup vs baseline: 1.4135x; 1.4135x over previous
"""Trainium2 Bass kernel for the AlignmentWithPrototype (vq_codebook) problem.

Math (matching the reference):
  x = projections.reshape(N, D); xn = l2norm(x); pn = l2norm(concat(lP, gP))
  sim = xn @ pn.T ; E = exp(sim/T)
  Sinkhorn(3 iters) keeps Q = E * a_n * b_k:
      a_n = 1/(N * sum_k E[n,k] b_k) ; b_k = 1/(K * sum_n E[n,k] a_n)  (K=2M=256)
  local/global assign = argmax_k E[n,k]*b_k over each half (a_n is row-constant)
  memory = gP[g_assign]; lin = memory @ W + bias; gated = lin_a * sigmoid(lin_b)
  out = l2norm(0.5*gated + 0.5*x)  == l2norm(gated + x)   (scale invariant)
  new_local = EMA last-write-wins scatter (done on host from local_assign)

Sharding: data-parallel over tokens, 8 cores x 16384 tokens. Column sums of the
Sinkhorn need a tiny (256 floats) AllReduce per iteration.

Precision choices: sim and E stay fp32 (argmax discrimination); the E^T spill
(row-sum side), the Sinkhorn `a` vector, and the whole GLU path run in bf16 —
their errors either average out (a, cs) or are far below output tolerance.
"""

import numpy as np

import concourse.bacc as bacc
import concourse.bass as bass
import concourse.mybir as mybir
import concourse.tile as tile


def _patch_act_tables():
    """Restrict the activation-table chooser to two sets (their positional
    ids are preserved; the others are emptied so the first-match chooser
    can't bounce between per-function sets, which costs ~1.3us per load)."""
    import concourse.hw_specs as hw_specs
    orig = hw_specs.get_activation_tables
    keep = {"natural_log_exp_and_others", "sigmoid_and_others"}

    def patched(module_arch):
        full = orig(module_arch)
        return {k: (v if k in keep else set()) for k, v in full.items()}

    bacc.get_activation_tables = patched


_patch_act_tables()

F32 = mybir.dt.float32
BF16 = mybir.dt.bfloat16
U32 = mybir.dt.uint32
U16 = mybir.dt.uint16
AF = mybir.ActivationFunctionType
ALU = mybir.AluOpType

D = 192
M = 128
K2 = 256  # 2*M
TEMP = 0.05
DECAY = 0.96
N_CORES = 8


def _pbcast(ap, p=128):
    """Broadcast a [1, F...] DRAM AP across p partitions (stride-0 partition
    dim; only legal as a DMA source)."""
    return bass.AP(tensor=ap.tensor, offset=ap.offset, ap=[[0, p]] + list(ap.ap[1:]))


def build_nc(N_s: int, with_bias: bool, n_cores: int = N_CORES):
    assert N_s % 512 == 0
    NT = N_s // 128      # token tiles
    NCH = N_s // 512     # chunks
    CPT = 4              # tiles per chunk
    N_total = N_s * n_cores

    nc = bacc.Bacc("TRN2", target_bir_lowering=False, debug=False,
                   num_devices=n_cores)

    x_d = nc.dram_tensor("x", [N_s, D], F32, kind="ExternalInput")
    pnT_d = nc.dram_tensor("pnT", [128, 2, K2], F32, kind="ExternalInput")
    pnT16_d = nc.dram_tensor("pnT16", [128, 2, K2], BF16, kind="ExternalInput")
    gP16_d = nc.dram_tensor("gP16", [128, K2], BF16, kind="ExternalInput")
    gluW16_d = nc.dram_tensor("gluW16", [128, 2, 2 * D], BF16, kind="ExternalInput")
    glub_d = nc.dram_tensor("glub", [1, 2 * D], F32, kind="ExternalInput")
    ident_d = nc.dram_tensor("ident", [128, 128], F32, kind="ExternalInput")
    iota_d = nc.dram_tensor("iota", [128, 1], F32, kind="ExternalInput")

    y_d = nc.dram_tensor("y", [N_s, D], F32, kind="ExternalOutput")
    lidx_d = nc.dram_tensor("lidx", [N_s], F32, kind="ExternalOutput")
    gidx_d = nc.dram_tensor("gidx", [N_s], F32, kind="ExternalOutput")

    ET_d = nc.dram_tensor("ET_spill", [K2, N_s], BF16)
    rs_d = nc.dram_tensor("rs_bounce", [N_s], F32)
    g_d = nc.dram_tensor("g_bounce", [N_s], F32)
    b_d = nc.dram_tensor("b_bounce", [1, K2], F32)
    cc_in = nc.dram_tensor("cc_in", [1, K2], F32)
    cc_out = nc.dram_tensor("cc_out", [n_cores, K2], F32, addr_space="Shared")
    V_d = nc.dram_tensor("V_spill", [N_s, D], F32)

    x_t = x_d[:].rearrange("(t p) d -> p t d", p=128)   # [128, NT, 192]
    y_t = y_d[:].rearrange("(t p) d -> p t d", p=128)
    v_t = V_d[:].rearrange("(t p) d -> p t d", p=128)

    with tile.TileContext(nc) as tc:
        with (
            tc.tile_pool(name="consts", bufs=1) as consts,
            tc.tile_pool(name="small", bufs=1) as small,
            tc.tile_pool(name="io", bufs=3) as io,
        ):
            pnT = consts.tile([128, 2, K2], F32)
            nc.sync.dma_start(pnT, pnT_d[:])
            pnT16 = consts.tile([128, 2, K2], BF16)
            nc.sync.dma_start(pnT16, pnT16_d[:])
            gP16 = consts.tile([128, K2], BF16)
            nc.sync.dma_start(gP16, gP16_d[:])
            gluW16 = consts.tile([128, 2, 2 * D], BF16)
            nc.sync.dma_start(gluW16, gluW16_d[:])
            glub = consts.tile([128, 2 * D], F32)
            nc.sync.dma_start(glub, _pbcast(glub_d[:]))
            ident = consts.tile([128, 128], F32)
            nc.sync.dma_start(ident, ident_d[:])
            iota = consts.tile([128, 1], F32)
            nc.sync.dma_start(iota, iota_d[:])
            ones8 = consts.tile([n_cores, 1], F32)
            nc.vector.memset(ones8, 1.0)
            ones1r = consts.tile([1, 128], F32)
            nc.vector.memset(ones1r, 1.0)

            ss = small.tile([128, NT], F32)       # row sum-of-squares
            rn = small.tile([128, NT], F32)       # 1/sqrt(ss)
            acol = small.tile([128, NT], F32)     # sinkhorn a (column layout)
            acol16 = small.tile([128, NT], BF16)
            rcol = small.tile([128, NT], F32)     # raw row sums (column layout)
            lcols = small.tile([128, NT], F32)    # local argmax (col layout)
            gcols = small.tile([128, NT], F32)
            vss = small.tile([128, NT], F32)      # output sumsq
            b128 = small.tile([128, K2], F32)     # final b bcast across partitions
            bcol16 = small.tile([128, 2, 1], BF16)
            ixl = small.tile([128, NT, 8], U32)   # top-8 local argmax indices
            ixg = small.tile([128, NT, 8], U32)   # top-8 global argmax indices
            magic = small.tile([128, 1], U32)
            nc.vector.memset(magic, 0x5f3759df)

            with tc.tile_pool(name="Epool", bufs=1) as Epool:
                E = Epool.tile([128, NT, K2], F32)

                # ------- Phase 1: normalize+transpose, sim, E, E^T ----------
                # iteration-1 Sinkhorn column sums accumulate during phase 1
                # (row sums with b=1 fall out of the exp accumulators)
                psCS_cm = tc.tile_pool(name="psCS", bufs=1, space="PSUM")
                psCS = psCS_cm.__enter__()
                ps_cs1 = psCS.tile([1, K2], F32, tag="cs1")
                with (
                    tc.tile_pool(name="p1w", bufs=3) as p1w,
                    tc.tile_pool(name="psT", bufs=1, space="PSUM") as psT,
                    tc.tile_pool(name="psS", bufs=2, space="PSUM") as psS,
                    tc.tile_pool(name="psX", bufs=1, space="PSUM") as psX,
                ):
                    pt_lo = psT.tile([128, 512], F32, tag="ptlo")
                    pt_hi = psT.tile([128, 512], F32, tag="pthi")
                    nc.vector.memset(pt_hi[64:128, :], 0.0)
                    for c in range(NCH):
                        xt = io.tile([128, CPT, D], F32, tag="xnat")
                        nc.sync.dma_start(xt, x_t[:, c * CPT:(c + 1) * CPT, :])
                        csl = slice(c * CPT, (c + 1) * CPT)
                        for t in range(CPT):
                            tt = c * CPT + t
                            scr = p1w.tile([128, D], F32, tag="scr")
                            nc.scalar.activation(
                                scr, xt[:, t, :], AF.Square,
                                accum_out=ss[:, tt:tt + 1],
                            )
                        # rn = exp(-0.5*ln(ss)) = 1/sqrt(ss)
                        nc.scalar.activation(rn[:, csl], ss[:, csl], AF.Ln)
                        nc.scalar.activation(rn[:, csl], rn[:, csl], AF.Exp,
                                             scale=-0.5)
                        # transpose+normalize: xT[:, tok] = x_tile.T @ diag(rn)
                        for t in range(CPT):
                            tt = c * CPT + t
                            dg = p1w.tile([128, 128], F32, tag="dg")
                            nc.vector.tensor_tensor(
                                dg, ident, rn[:, tt:tt + 1].to_broadcast((128, 128)),
                                op=ALU.mult,
                            )
                            nc.tensor.matmul(
                                pt_lo[:, t * 128:(t + 1) * 128], xt[:, t, 0:128], dg,
                                start=True, stop=True,
                            )
                            nc.tensor.matmul(
                                pt_hi[0:64, t * 128:(t + 1) * 128], xt[:, t, 128:192], dg,
                                start=True, stop=True,
                            )
                        xT = p1w.tile([128, 2, 512], F32, tag="xT")
                        nc.vector.tensor_copy(xT[:, 0, :], pt_lo)
                        nc.vector.tensor_copy(xT[:, 1, :], pt_hi)
                        xT16 = p1w.tile([128, 2, 512], BF16, tag="xT16")
                        nc.vector.tensor_copy(xT16[:, 0, :], pt_lo)
                        nc.vector.tensor_copy(xT16[:, 1, :], pt_hi)
                        # sim (token-major, fp32) -> E
                        for t in range(CPT):
                            tt = c * CPT + t
                            ps = psS.tile([128, K2], F32, tag="sim")
                            nc.tensor.matmul(
                                ps, xT[:, 0, t * 128:(t + 1) * 128], pnT[:, 0, :],
                                start=True, stop=False,
                            )
                            nc.tensor.matmul(
                                ps, xT[:, 1, t * 128:(t + 1) * 128], pnT[:, 1, :],
                                start=False, stop=True,
                            )
                            nc.scalar.activation(
                                E[:, tt, :], ps, AF.Exp, scale=1.0 / TEMP,
                                accum_out=rcol[:, tt:tt + 1],
                            )
                        # iteration-1 a-columns and cs accumulation
                        nc.vector.tensor_scalar_mul(
                            acol[:, csl], rcol[:, csl], float(N_total)
                        )
                        nc.vector.reciprocal(acol[:, csl], acol[:, csl])
                        nc.vector.tensor_copy(acol16[:, csl], acol[:, csl])
                        E16v_ = E.bitcast(BF16)
                        for t in range(CPT):
                            tt = c * CPT + t
                            nc.tensor.matmul(
                                ps_cs1, acol16[:, tt:tt + 1], E16v_[:, tt, 1::2],
                                start=(tt == 0), stop=(tt == NT - 1),
                            )
                        # simT (proto-major, bf16) -> E^T (bf16) -> DRAM
                        ps1 = psX.tile([128, 512], F32, tag="sT1")
                        ps2 = psX.tile([128, 512], F32, tag="sT2")
                        nc.tensor.matmul(ps1, pnT16[:, 0, 0:128], xT16[:, 0, :],
                                         start=True, stop=False)
                        nc.tensor.matmul(ps1, pnT16[:, 1, 0:128], xT16[:, 1, :],
                                         start=False, stop=True)
                        nc.tensor.matmul(ps2, pnT16[:, 0, 128:256], xT16[:, 0, :],
                                         start=True, stop=False)
                        nc.tensor.matmul(ps2, pnT16[:, 1, 128:256], xT16[:, 1, :],
                                         start=False, stop=True)
                        et16 = p1w.tile([128, 2, 512], BF16, tag="et16")
                        nc.scalar.activation(et16[:, 0, :], ps1, AF.Exp, scale=1.0 / TEMP)
                        nc.scalar.activation(et16[:, 1, :], ps2, AF.Exp, scale=1.0 / TEMP)
                        sl = slice(c * 512, (c + 1) * 512)
                        nc.sync.dma_start(ET_d[0:128, sl], et16[:, 0, :])
                        nc.sync.dma_start(ET_d[128:256, sl], et16[:, 1, :])

                # ---------------- Phase 2: Sinkhorn iterations -------------
                nc.vector.memset(bcol16, 1.0)
                with (
                    tc.tile_pool(name="p2w", bufs=3) as p2w,
                    tc.tile_pool(name="etp", bufs=8) as etp,
                    tc.tile_pool(name="psR", bufs=3, space="PSUM") as psR,
                    tc.tile_pool(name="psC", bufs=1, space="PSUM") as psC,
                ):
                    N_ITERS = 2
                    for it in range(N_ITERS):
                        if it > 0:
                            # row sums with current b (iteration-1 rs and cs
                            # were already folded into phase 1)
                            for c in range(NCH):
                                et = etp.tile([128, 2, 512], BF16, tag="et")
                                sl = slice(c * 512, (c + 1) * 512)
                                sl_a = slice(c * 512, c * 512 + 256)
                                sl_b = slice(c * 512 + 256, (c + 1) * 512)
                                eng0 = [nc.sync, nc.scalar][c % 2]
                                eng1 = [nc.scalar, nc.sync][c % 2]
                                eng0.dma_start(et[:, 0, 0:256], ET_d[0:128, sl_a])
                                eng1.dma_start(et[:, 0, 256:512], ET_d[0:128, sl_b])
                                eng1.dma_start(et[:, 1, 0:256], ET_d[128:256, sl_a])
                                eng0.dma_start(et[:, 1, 256:512], ET_d[128:256, sl_b])
                                ps_rs = psR.tile([1, 512], F32, tag="rs")
                                nc.tensor.matmul(
                                    ps_rs, bcol16[:, 0, :], et[:, 0, :],
                                    start=True, stop=False,
                                )
                                nc.tensor.matmul(
                                    ps_rs, bcol16[:, 1, :], et[:, 1, :],
                                    start=False, stop=True,
                                )
                                rsr = p2w.tile([1, 512], F32, tag="rsr")
                                nc.scalar.copy(rsr, ps_rs)
                                nc.sync.dma_start(rs_d[sl], rsr)
                            # read back in column layout [p, t] (strided
                            # gather, split across DMA queues)
                            rs_v = rs_d[:].rearrange("(t p) -> p t", p=128)
                            q4 = NT // 4
                            for qq in range(4):
                                nc.sync.dma_start(
                                    rcol[:, qq * q4:(qq + 1) * q4],
                                    rs_v[:, qq * q4:(qq + 1) * q4],
                                )
                        if it == 0:
                            ps_cs = ps_cs1
                        else:
                            # a = 1/(N_total * rs)
                            nc.vector.tensor_scalar_mul(acol, rcol, float(N_total))
                            nc.vector.reciprocal(acol, acol)
                            nc.vector.tensor_copy(acol16, acol)
                            # cs[k] = sum_n a_n E[n,k] in bf16: read the high
                            # half-words of fp32 E as a strided bf16 view
                            # (truncation bias is uniform and cancels in b)
                            E16v = E.bitcast(BF16)
                            ps_cs = psC.tile([1, K2], F32, tag="cs")
                            for tt in range(NT):
                                nc.tensor.matmul(
                                    ps_cs, acol16[:, tt:tt + 1],
                                    E16v[:, tt, 1::2],
                                    start=(tt == 0), stop=(tt == NT - 1),
                                )
                        csrow = p2w.tile([1, K2], F32, tag="csrow")
                        nc.vector.tensor_copy(csrow, ps_cs)
                        nc.sync.dma_start(cc_in[:], csrow)
                        nc.gpsimd.collective_compute(
                            "AllGather",
                            ALU.bypass,
                            replica_groups=[list(range(n_cores))],
                            ins=[cc_in[:]],
                            outs=[cc_out[:]],
                        )
                        csg = p2w.tile([n_cores, K2], F32, tag="csg")
                        nc.sync.dma_start(csg, cc_out[:])
                        ps_bs = psC.tile([1, K2], F32, tag="bs")
                        nc.tensor.matmul(ps_bs, ones8, csg, start=True, stop=True)
                        bsb = p2w.tile([1, K2], F32, tag="bsb")
                        nc.vector.tensor_scalar_mul(bsb, ps_bs, float(K2))
                        nc.vector.reciprocal(bsb, bsb)
                        if it < N_ITERS - 1:
                            # b columns via PE transpose of the two halves
                            ps_bt = psC.tile([128, 2], F32, tag="bt")
                            nc.tensor.transpose(ps_bt[:, 0:1], bsb[:, 0:128],
                                                ones1r[:, 0:1])
                            nc.tensor.transpose(ps_bt[:, 1:2], bsb[:, 128:256],
                                                ones1r[:, 0:1])
                            nc.vector.tensor_copy(
                                bcol16, ps_bt[:].rearrange("p (c o) -> p c o", o=1)
                            )
                        else:
                            # b128 = ones ⊗ b via a rank-1 matmul broadcast
                            ps_bb = psC.tile([128, K2], F32, tag="bb")
                            nc.tensor.matmul(ps_bb, ones1r, bsb, start=True, stop=True)
                            nc.vector.tensor_copy(b128, ps_bb)

                psCS_cm.__exit__(None, None, None)

                # ------- Phase 3+4a fused: argmax -> gather -> GLU -> v -----
                with (
                    tc.tile_pool(name="p4w", bufs=3) as p4w,
                    tc.tile_pool(name="vtp", bufs=4) as vtp,
                    tc.tile_pool(name="psI", bufs=2, space="PSUM") as psI,
                    tc.tile_pool(name="psM", bufs=1, space="PSUM") as psM,
                    tc.tile_pool(name="psL", bufs=2, space="PSUM") as psL,
                ):
                    GRP = 1
                    for g in range(NCH // GRP):
                        vts = []
                        for c4 in range(GRP):
                            c = g * GRP + c4
                            csl4 = slice(c * CPT, (c + 1) * CPT)
                            for t in range(CPT):
                                tt = c * CPT + t
                                qt = p4w.tile([128, K2], F32, tag="qt")
                                nc.gpsimd.tensor_tensor(qt, E[:, tt, :], b128,
                                                        op=ALU.mult)
                                vm = p4w.tile([128, 8], F32, tag="vm")
                                nc.vector.max(vm, qt[:, 0:128])
                                nc.vector.max_index(ixl[:, tt, :], vm, qt[:, 0:128])
                                vm2 = p4w.tile([128, 8], F32, tag="vm2")
                                nc.vector.max(vm2, qt[:, 128:256])
                                nc.vector.max_index(ixg[:, tt, :], vm2, qt[:, 128:256])
                            # extract this chunk's gidx as f32 columns (strided)
                            nc.vector.tensor_copy(gcols[:, csl4],
                                                  ixg[:, csl4, 0])
                            # bounce this chunk's gidx to token-major row in DRAM
                            pig = psI.tile([CPT, 128], F32, tag="pig")
                            nc.tensor.transpose(pig, gcols[:, csl4], ident)
                            gTc = p4w.tile([CPT, 128], F32, tag="gTc")
                            nc.scalar.copy(gTc, pig)
                            gsl = slice(c * 512, (c + 1) * 512)
                            nc.sync.dma_start(
                                g_d[gsl].rearrange("(t p) -> t p", p=128), gTc
                            )
                            g128 = p4w.tile([128, 512], F32, tag="g128")
                            nc.sync.dma_start(
                                g128,
                                _pbcast(g_d[:].rearrange("(a f) -> a f", f=512)[c:c + 1, :]),
                            )
                            oh = p4w.tile([128, 512], BF16, tag="oh")
                            nc.vector.tensor_tensor(
                                oh, iota[:].to_broadcast((128, 512)), g128,
                                op=ALU.is_equal,
                            )
                            ps_ma1 = psM.tile([128, 512], F32, tag="ma1")
                            ps_ma2 = psM.tile([128, 512], F32, tag="ma2")
                            nc.tensor.matmul(ps_ma1, gP16[:, 0:128], oh, start=True, stop=True)
                            nc.tensor.matmul(ps_ma2, gP16[:, 128:256], oh, start=True, stop=True)
                            maT = p4w.tile([128, 2, 512], BF16, tag="maT")
                            nc.scalar.copy(maT[:, 0, :], ps_ma1)
                            nc.scalar.copy(maT[:, 1, :], ps_ma2)
                            x2 = io.tile([128, CPT, D], F32, tag="xnat")
                            nc.sync.dma_start(x2, x_t[:, c * CPT:(c + 1) * CPT, :])
                            vt = vtp.tile([128, CPT, D], F32, tag="vt")
                            vts.append(vt)
                            for t in range(CPT):
                                tt = c * CPT + t
                                tsl = slice(t * 128, (t + 1) * 128)
                                ps_lin = psL.tile([128, 2 * D], F32, tag="lin")
                                nc.tensor.matmul(ps_lin, maT[:, 0, tsl], gluW16[:, 0, :],
                                                 start=True, stop=False)
                                nc.tensor.matmul(ps_lin, maT[:, 1, tsl], gluW16[:, 1, :],
                                                 start=False, stop=True)
                                if with_bias:
                                    lsb = p4w.tile([128, 2 * D], F32, tag="lsb")
                                    nc.vector.tensor_tensor(
                                        lsb, ps_lin, glub, op=ALU.add
                                    )
                                    lin_src = lsb
                                else:
                                    lin_src = ps_lin
                                sg = p4w.tile([128, D], F32, tag="sg")
                                nc.scalar.activation(sg, lin_src[:, D:2 * D], AF.Sigmoid)
                                nc.vector.tensor_tensor(
                                    vt[:, t, :], lin_src[:, 0:D], sg, op=ALU.mult
                                )
                                nc.gpsimd.tensor_tensor(
                                    vt[:, t, :], vt[:, t, :], x2[:, t, :], op=ALU.add
                                )
                                scr2 = p4w.tile([128, D], F32, tag="scr2")
                                nc.scalar.activation(
                                    scr2, vt[:, t, :], AF.Square,
                                    accum_out=vss[:, tt:tt + 1],
                                )
                        # rnorm = rsqrt(vss) for the whole group:
                        # magic seed + 2 Newton steps (batched [128, 16])
                        gw = GRP * CPT
                        sgrp = vss[:, g * gw:(g + 1) * gw]
                        rg = p4w.tile([128, gw], F32, tag="rg")
                        rgu = rg.bitcast(U32)
                        nc.vector.tensor_scalar(
                            rgu, sgrp.bitcast(U32), 1, None,
                            op0=ALU.logical_shift_right,
                        )
                        nc.vector.tensor_tensor(
                            rgu, magic[:].to_broadcast((128, gw)), rgu,
                            op=ALU.subtract,
                        )
                        t1 = p4w.tile([128, gw], F32, tag="t1")
                        for _ in range(2):
                            nc.vector.tensor_tensor(t1, rg, rg, op=ALU.mult)
                            nc.vector.tensor_tensor(t1, t1, sgrp, op=ALU.mult)
                            nc.vector.tensor_scalar(
                                t1, t1, -0.5, 1.5, op0=ALU.mult, op1=ALU.add
                            )
                            nc.vector.tensor_tensor(rg, rg, t1, op=ALU.mult)
                        for c4 in range(GRP):
                            c = g * GRP + c4
                            yt = p4w.tile([128, CPT, D], F32, tag="yt")
                            for t in range(CPT):
                                nc.vector.tensor_scalar_mul(
                                    yt[:, t, :], vts[c4][:, t, :],
                                    rg[:, c4 * CPT + t:c4 * CPT + t + 1]
                                )
                            nc.sync.dma_start(y_t[:, c * CPT:(c + 1) * CPT, :], yt)
                    # local argmax columns -> token-major output (host uses it)
                    nc.vector.tensor_copy(lcols, ixl[:, :, 0])
                    pil = psI.tile([NT, 128], F32, tag="pig")
                    nc.tensor.transpose(pil, lcols, ident)
                    lT = p4w.tile([NT, 128], F32, tag="lT")
                    nc.vector.tensor_copy(lT, pil)
                    nc.sync.dma_start(
                        lidx_d[:].rearrange("(t p) -> t p", p=128), lT
                    )
                    pig2 = psI.tile([NT, 128], F32, tag="pig")
                    nc.tensor.transpose(pig2, gcols, ident)
                    gT = p4w.tile([NT, 128], F32, tag="gT")
                    nc.vector.tensor_copy(gT, pig2)
                    nc.sync.dma_start(
                        gidx_d[:].rearrange("(t p) -> t p", p=128), gT
                    )


    nc.compile()
    return nc


def _host_consts(localPrototypes, globalPrototypes, glu_W, glu_b):
    protos = np.concatenate([localPrototypes, globalPrototypes], axis=0).astype(np.float32)
    sq = np.maximum((protos * protos).sum(axis=1, keepdims=True), 1e-12)
    pn = protos / np.sqrt(sq)

    pnT_pad = np.zeros((256, K2), np.float32)
    pnT_pad[0:D] = pn.T
    pnT_pad = np.ascontiguousarray(pnT_pad.reshape(2, 128, K2).transpose(1, 0, 2))

    import ml_dtypes
    bf16 = ml_dtypes.bfloat16

    gP_pad = np.zeros((128, K2), np.float32)
    gP_pad[:, 0:D] = globalPrototypes.astype(np.float32)

    gluW_pad = np.zeros((256, 2 * D), np.float32)
    gluW_pad[0:D] = glu_W.astype(np.float32)
    gluW_pad = np.ascontiguousarray(gluW_pad.reshape(2, 128, 2 * D).transpose(1, 0, 2))

    return {
        "pnT": pnT_pad,
        "pnT16": pnT_pad.astype(bf16),
        "gP16": gP_pad.astype(bf16),
        "gluW16": gluW_pad.astype(bf16),
        "glub": np.ascontiguousarray(glu_b.astype(np.float32).reshape(1, 2 * D)),
        "ident": np.eye(128, dtype=np.float32),
        "iota": np.arange(128, dtype=np.float32).reshape(128, 1),
    }


_NC_CACHE = {}


def _get_nc(N_s, with_bias):
    key = (N_s, with_bias)
    if key not in _NC_CACHE:
        _NC_CACHE[key] = build_nc(N_s, with_bias)
    return _NC_CACHE[key]


def kernel(projections, localPrototypes, globalPrototypes, glu_W, glu_b):
    from concourse.bass_utils import run_bass_kernel_spmd

    B, T, d = projections.shape
    N = B * T
    N_s = N // N_CORES
    x_flat = np.ascontiguousarray(projections.astype(np.float32).reshape(N, d))

    with_bias = bool(np.any(glu_b))
    consts = _host_consts(localPrototypes, globalPrototypes, glu_W, glu_b)
    nc = _get_nc(N_s, with_bias)

    in_maps = []
    for c in range(N_CORES):
        m = {"x": x_flat[c * N_s:(c + 1) * N_s]}
        m.update(consts)
        in_maps.append(m)

    res = run_bass_kernel_spmd(nc, in_maps, list(range(N_CORES)))
    results = res.results

    y = np.concatenate([r["y"] for r in results], axis=0)
    normed = y.reshape(B, T, d)

    lidx = np.concatenate([r["lidx"] for r in results], axis=0)
    lidx = np.rint(lidx).astype(np.int64)

    # EMA scatter, last-write-wins
    last = np.full(M, -1, dtype=np.int64)
    last[lidx] = np.arange(N, dtype=np.int64)
    lP = localPrototypes.astype(np.float32)
    new_local = lP.copy()
    valid = last >= 0
    new_local[valid] = DECAY * lP[valid] + np.float32(1.0 - DECAY) * x_flat[last[valid]]

    return normed, new_local
